# revision 28
# baseline (speedup 1.0000x reference)
"""GQA (H=32, KV=8, D=128, T=2048, hid=4096) causal attention + RoPE,
tensor-parallel over heads across 8 NeuronCores.

v2 design (bf16 operands, fp32 PSUM accumulation):
  - Core i owns kv-head i and query heads 4i..4i+3.
  - wq/wk/wv column-sharded; wo ROW-sharded [512, 4096]; x shipped
    pre-transposed in bf16 [hid, T].
  - Phase 1: Q_T/K_T/V_T projections ([d, t] layout), RoPE fused into the
    epilogue using a DVE stream_shuffle for rotate-half (no SBUF DMAs).
  - Phase 2 per 512-query chunk: causal attention in S_T [kt, qt] layout,
    unnormalized softmax (exp fp32-safe), denominator via ones-matmul,
    diagonal kt-tiles narrowed to their live query range (saves PE rows),
    single [128,128] triangle mask reused for every diagonal block.
  - Per chunk: partial o_proj out_part[t, 4096] = att_chunk @ wo_rows from
    SBUF (no DRAM round trip), then a bf16 ReduceScatter over the 8 cores
    that overlaps with the next chunk's attention.
Host assembles the 8 cores' ReduceScatter shards into the full output.
"""

import math
import numpy as np
import ml_dtypes

import concourse.bass as bass
import concourse.mybir as mybir
import concourse.tile as tile
from concourse import bacc
from concourse.bass_utils import run_bass_kernel_spmd

T = 2048
HID = 4096
H = 32
KV = 8
D = 128
NC = 8
HQ = H // NC          # 4 query heads per core
DQ = HQ * D           # 512
KT = HID // 128       # 32 contraction tiles
TC = T // 512         # 4 t-chunks
ROPE_BASE = 10000.0

BF16 = mybir.dt.bfloat16
F32 = mybir.dt.float32
NPBF16 = np.dtype(ml_dtypes.bfloat16)

_BUILD_CACHE = {}
RUN_KWARGS = {}  # test harness hook (e.g. {"trace": True})


def _build_nc():
    nc = bacc.Bacc(None, target_bir_lowering=False, num_devices=NC)

    xT = nc.declare_dram_parameter("xT", [HID, T], BF16, isOutput=False)
    wq = nc.declare_dram_parameter("wq", [HID, DQ], BF16, isOutput=False)
    wk = nc.declare_dram_parameter("wk", [HID, D], BF16, isOutput=False)
    wv = nc.declare_dram_parameter("wv", [HID, D], BF16, isOutput=False)
    wo = nc.declare_dram_parameter("wo", [DQ, HID], BF16, isOutput=False)
    cosT = nc.declare_dram_parameter("cosT", [D, T], F32, isOutput=False)
    sinT = nc.declare_dram_parameter("sinT", [D, T], F32, isOutput=False)  # sign-folded
    tri = nc.declare_dram_parameter("tri", [128, 128], BF16, isOutput=False)
    ones = nc.declare_dram_parameter("ones", [128, 1], BF16, isOutput=False)
    ident = nc.declare_dram_parameter("ident", [128, 128], BF16, isOutput=False)
    # ReduceScatter shards: chunk c -> rows [64c, 64(c+1))
    out = nc.declare_dram_parameter("out", [TC * (512 // NC), HID], BF16,
                                    isOutput=True)

    rs_in = [nc.dram_tensor(f"rs_in{c}", [512, HID], BF16) for c in range(TC)]
    rs_out = [nc.dram_tensor(f"rs_out{c}", [512 // NC, HID], BF16)
              for c in range(TC)]

    inv_sqrt_d = 1.0 / math.sqrt(D)

    with tile.TileContext(nc) as tc:
        with tc.tile_pool(name="persist", bufs=1) as pp:
            qt_sb = [pp.tile([128, T], BF16, tag=f"qt{h}", name=f"qt{h}")
                     for h in range(HQ)]
            kt_sb = pp.tile([128, T], BF16, tag="kt")
            vt_sb = pp.tile([128, T], BF16, tag="vt")        # V transposed [d, t]
            vn_sb = pp.tile([128, T], BF16, tag="vn")        # V natural [t, d] x16
            cos_sb = pp.tile([128, T], F32, tag="cos")
            sin_sb = pp.tile([128, T], F32, tag="sin")
            tri_sb = pp.tile([128, 128], BF16, tag="tri")
            ones_sb = pp.tile([128, 1], BF16, tag="ones")
            id_sb = pp.tile([128, 128], BF16, tag="ident")
            wo_sb = pp.tile([128, HQ * HID], BF16, tag="wo")

            # consts ride the idle gpsimd SWDGE queue (needed by ~60us)
            nc.gpsimd.dma_start(cos_sb[:, :], cosT[:, :])
            nc.gpsimd.dma_start(sin_sb[:, :], sinT[:, :])
            nc.gpsimd.dma_start(tri_sb[:, :], tri[:, :])
            nc.gpsimd.dma_start(ones_sb[:, :], ones[:, :])
            nc.gpsimd.dma_start(id_sb[:, :], ident[:, :])

            def load_consts_and_wo():
                # issued AFTER wq/wk/wv on the scalar queue: wo isn't needed
                # until the first o_proj (~250us), keep it off the startup path
                for a in range(HQ):
                    nc.scalar.dma_start(
                        wo_sb[:, a * HID:(a + 1) * HID],
                        wo[a * 128:(a + 1) * 128, :])

            _phase1_qkv(nc, tc, xT, wq, wk, wv,
                        qt_sb, kt_sb, vt_sb, vn_sb, cos_sb, sin_sb, id_sb,
                        load_consts_and_wo)

            _phase23(nc, tc, qt_sb, kt_sb, vn_sb, tri_sb, ones_sb, wo_sb,
                     rs_in, rs_out, out, inv_sqrt_d)

    nc.compile()
    return nc


def _phase1_qkv(nc, tc, xT, wq, wk, wv,
                qt_sb, kt_sb, vt_sb, vn_sb, cos_sb, sin_sb, id_sb,
                load_consts_and_wo):
    with tc.tile_pool(name="wqkv", bufs=1) as wp:
        wq_sb = wp.tile([128, KT * DQ], BF16, tag="wq")
        wk_sb = wp.tile([128, KT * D], BF16, tag="wk")
        wv_sb = wp.tile([128, KT * D], BF16, tag="wv")
        # per-k-tile loads on the scalar queue so the k=0 matmuls start as
        # soon as slice 0 lands (xt streaming owns the sync queue)
        for k in range(KT):
            nc.scalar.dma_start(
                wq_sb[:, k * DQ:(k + 1) * DQ], wq[k * 128:(k + 1) * 128, :])
            nc.scalar.dma_start(
                wk_sb[:, k * D:(k + 1) * D], wk[k * 128:(k + 1) * 128, :])
            nc.scalar.dma_start(
                wv_sb[:, k * D:(k + 1) * D], wv[k * 128:(k + 1) * 128, :])
        load_consts_and_wo()

        with (
            tc.tile_pool(name="xrhs", bufs=4) as xp,
            tc.tile_pool(name="qkvps", bufs=1, space="PSUM") as qps,
            tc.tile_pool(name="ropetmp", bufs=2) as rp,
            tc.tile_pool(name="vtp", bufs=2, space="PSUM") as vps,
        ):
            for tcn in range(TC):
                ts = tcn * 512
                pq = [qps.tile([128, 512], F32, tag=f"pq{h}", name=f"pq{h}")
                      for h in range(HQ)]
                pk = qps.tile([128, 512], F32, tag="pk")
                pv = qps.tile([128, 512], F32, tag="pv")
                for k in range(KT):
                    xt = xp.tile([128, 512], BF16, tag="xt")
                    nc.sync.dma_start(
                        xt[:, :], xT[k * 128:(k + 1) * 128, ts:ts + 512])
                    for h in range(HQ):
                        nc.tensor.matmul(
                            pq[h][:, :],
                            wq_sb[:, k * DQ + h * 128: k * DQ + (h + 1) * 128],
                            xt[:, :],
                            start=(k == 0), stop=(k == KT - 1),
                        )
                    nc.tensor.matmul(
                        pk[:, :], wk_sb[:, k * D:(k + 1) * D], xt[:, :],
                        start=(k == 0), stop=(k == KT - 1))
                    nc.tensor.matmul(
                        pv[:, :], wv_sb[:, k * D:(k + 1) * D], xt[:, :],
                        start=(k == 0), stop=(k == KT - 1))

                # epilogue: RoPE for q heads + k (stream_shuffle rotate-half)
                # evacuate all psum tiles first (frees banks for next chunk),
                # then RoPE math runs from SBUF off the critical path
                qn_ts = []
                for h in range(HQ + 1):
                    src = pq[h] if h < HQ else pk
                    qn_t = rp.tile([128, 512], F32, tag=f"qnat{h}",
                                   name=f"qnat{h}")
                    if h % 2 == 0:
                        nc.scalar.copy(qn_t[:, :], src[:, :])
                    else:
                        nc.vector.tensor_copy(qn_t[:, :], src[:, :])
                    qn_ts.append(qn_t)
                nc.scalar.copy(vt_sb[:, ts:ts + 512], pv[:, :])
                # V transpose for this chunk's four 128-tiles, interleaved
                # with the next chunk's projection matmuls on the PE
                for u in range(4):
                    t16 = 4 * tcn + u
                    vp = vps.tile([128, 128], BF16, tag="vtp")
                    nc.tensor.transpose(
                        vp[:, :], vt_sb[:, t16 * 128:(t16 + 1) * 128],
                        id_sb[:, :])
                    if u % 2 == 0:
                        nc.scalar.copy(
                            vn_sb[:, t16 * 128:(t16 + 1) * 128], vp[:, :])
                    else:
                        nc.vector.tensor_copy(
                            vn_sb[:, t16 * 128:(t16 + 1) * 128], vp[:, :])
                for h in range(HQ + 1):
                    qn_t = qn_ts[h]
                    dst = qt_sb[h] if h < HQ else kt_sb
                    sh_t = rp.tile([128, 512], F32, tag="qshuf")
                    nc.scalar.dma_start(sh_t[0:64, :], qn_t[64:128, :])
                    nc.scalar.dma_start(sh_t[64:128, :], qn_t[0:64, :])
                    ss_t = rp.tile([128, 512], F32, tag="qsin")
                    nc.gpsimd.tensor_tensor(
                        ss_t[:, :], sh_t[:, :], sin_sb[:, ts:ts + 512],
                        op=mybir.AluOpType.mult)
                    qc_t = rp.tile([128, 512], F32, tag="qcos")
                    nc.vector.tensor_tensor(
                        qc_t[:, :], qn_t[:, :], cos_sb[:, ts:ts + 512],
                        op=mybir.AluOpType.mult)
                    eng = nc.vector if (h % 2 == 0) else nc.gpsimd
                    eng.tensor_tensor(
                        dst[:, ts:ts + 512], qc_t[:, :], ss_t[:, :],
                        op=mybir.AluOpType.add)




def _phase23(nc, tc, qt_sb, kt_sb, vn_sb, tri_sb, ones_sb, wo_sb,
             rs_in, rs_out, out, inv_sqrt_d):
    with (
        tc.tile_pool(name="attn", bufs=4) as ap,
        tc.tile_pool(name="attops", bufs=2, space="PSUM") as sps,
        tc.tile_pool(name="attacc", bufs=2, space="PSUM") as acc_ps,
        tc.tile_pool(name="attout", bufs=3) as aop,
        tc.tile_pool(name="atchunk", bufs=2) as atp,
        tc.tile_pool(name="oproj", bufs=2, space="PSUM") as ops,
        tc.tile_pool(name="ostage", bufs=3) as osp,
    ):
        for qc in range(TC):
            qs = qc * 512
            at_t = atp.tile([128, HQ * 512], BF16, tag="atc")  # [d, h*q]
            for h in range(HQ):
                den_ps = acc_ps.tile([1, 512], F32, tag="den")
                o_ps = acc_ps.tile([128, 512], F32, tag="opv")
                n_kt = 4 * qc + 4

                def tile_geom(kt):
                    m = kt - 4 * qc          # >=0: diagonal block index
                    off = 128 * m if m > 0 else 0   # live query range start
                    return m, off, 512 - off

                # 1-step software pipeline: S(kt+1) issues before den/PV(kt)
                # so the PE never waits on the exp between S and den/PV.
                e_ts = [None] * n_kt

                def s_exp(kt):
                    m, off, w = tile_geom(kt)
                    s_ps = sps.tile([128, 512], F32, tag="st")
                    nc.tensor.matmul(
                        s_ps[:, :w],
                        kt_sb[:, kt * 128:(kt + 1) * 128],
                        qt_sb[h][:, qs + off:qs + 512],
                        start=True, stop=True, skip_group_check=True)
                    e_t = ap.tile([128, 512], BF16, tag="et")
                    nc.scalar.activation(
                        e_t[:, :w], s_ps[:, :w],
                        mybir.ActivationFunctionType.Exp,
                        scale=inv_sqrt_d)
                    if m >= 0:  # mask the triangular boundary sub-tile
                        nc.vector.tensor_tensor(
                            e_t[:, :128], e_t[:, :128], tri_sb[:, :],
                            op=mybir.AluOpType.mult)
                    e_ts[kt] = e_t

                def den_pv(kt):
                    m, off, w = tile_geom(kt)
                    e_t = e_ts[kt]
                    nc.tensor.matmul(
                        den_ps[:, off:512], ones_sb[:, :], e_t[:, :w],
                        start=(kt == 0), stop=(kt == n_kt - 1),
                        skip_group_check=True)
                    nc.tensor.matmul(
                        o_ps[:, off:512],
                        vn_sb[:, kt * 128:(kt + 1) * 128],
                        e_t[:, :w],
                        start=(kt == 0), stop=(kt == n_kt - 1),
                        skip_group_check=True)
                    e_ts[kt] = None

                s_exp(0)
                for kt in range(1, n_kt):
                    s_exp(kt)
                    den_pv(kt - 1)
                den_pv(n_kt - 1)
                rc_t = aop.tile([1, 512], F32, tag="recip")
                nc.vector.reciprocal(rc_t[:, :], den_ps[:, :])
                rb_t = aop.tile([128, 512], F32, tag="recipb")
                nc.gpsimd.partition_broadcast(rb_t[:, :], rc_t[0:1, :])
                nc.vector.tensor_tensor(
                    at_t[:, h * 512:(h + 1) * 512], o_ps[:, :], rb_t[:, :],
                    op=mybir.AluOpType.mult)

            # partial o_proj for this chunk: out_part[t, n] from SBUF
            for tt in range(4):
                stage = osp.tile([128, HID], BF16, tag="ostage")
                for nc8 in range(HID // 512):
                    op_ps = ops.tile([128, 512], F32, tag="ops")
                    for h in range(HQ):
                        nc.tensor.matmul(
                            op_ps[:, :],
                            at_t[:, h * 512 + tt * 128:h * 512 + (tt + 1) * 128],
                            wo_sb[:, h * HID + nc8 * 512:h * HID + (nc8 + 1) * 512],
                            start=(h == 0), stop=(h == HQ - 1))
                    if nc8 % 2 == 0:
                        nc.scalar.copy(
                            stage[:, nc8 * 512:(nc8 + 1) * 512], op_ps[:, :])
                    else:
                        nc.vector.tensor_copy(
                            stage[:, nc8 * 512:(nc8 + 1) * 512], op_ps[:, :])
                nc.sync.dma_start(
                    rs_in[qc][tt * 128:(tt + 1) * 128, :], stage[:, :])
                if qc == TC - 1:
                    # last chunk: per-t-tile RS so the exposed tail is one
                    # 1MB collective instead of a 4MB one
                    nc.gpsimd.collective_compute(
                        "ReduceScatter",
                        mybir.AluOpType.add,
                        replica_groups=[list(range(NC))],
                        ins=[rs_in[qc][tt * 128:(tt + 1) * 128, :]],
                        outs=[rs_out[qc][tt * 16:(tt + 1) * 16, :]],
                    )

            if qc < TC - 1:
                nc.gpsimd.collective_compute(
                    "ReduceScatter",
                    mybir.AluOpType.add,
                    replica_groups=[list(range(NC))],
                    ins=[rs_in[qc][:, :]],
                    outs=[rs_out[qc][:, :]],
                )

        # deferred: drain RS shards to the output param (waits on collectives,
        # so keep it off the queues that feed the compute pipeline)
        for qc in range(TC):
            nc.gpsimd.dma_start(out[qc * 64:(qc + 1) * 64, :], rs_out[qc][:, :])


def _host_consts():
    inv = 1.0 / (ROPE_BASE ** (np.arange(0, D, 2, dtype=np.float32) / D))
    t = np.arange(T, dtype=np.float32)
    f = np.outer(t, inv)
    e = np.concatenate([f, f], axis=-1)
    cos = np.cos(e).astype(np.float32)
    sin = np.sin(e).astype(np.float32)
    sgn = np.where(np.arange(D) < D // 2, -1.0, 1.0).astype(np.float32)
    cosT = np.ascontiguousarray(cos.T)
    sinT = np.ascontiguousarray((sin * sgn).T)
    # triangle mask for diagonal 128x128 sub-tiles: keep iff col >= row
    p = np.arange(128)[:, None]
    fr = np.arange(128)[None, :]
    tri = (fr >= p).astype(NPBF16)
    ones = np.ones((128, 1), NPBF16)
    ident = np.eye(128, dtype=np.float32).astype(NPBF16)
    return cosT, sinT, tri, ones, ident


def kernel(x, wq, wk, wv, wo, mask=None, **_ignored):
    x = np.asarray(x, dtype=np.float32)
    B = x.shape[0]
    xT = np.ascontiguousarray(x.reshape(T, HID).T.astype(NPBF16))
    wq16 = np.asarray(wq, dtype=np.float32).astype(NPBF16)
    wk16 = np.asarray(wk, dtype=np.float32).astype(NPBF16)
    wv16 = np.asarray(wv, dtype=np.float32).astype(NPBF16)
    wo16 = np.asarray(wo, dtype=np.float32).astype(NPBF16)
    cosT, sinT, tri, ones, ident = _host_consts()

    if "nc" not in _BUILD_CACHE:
        _BUILD_CACHE["nc"] = _build_nc()
    nc = _BUILD_CACHE["nc"]

    in_maps = []
    for i in range(NC):
        in_maps.append({
            "xT": xT,
            "wq": np.ascontiguousarray(wq16[:, i * DQ:(i + 1) * DQ]),
            "wk": np.ascontiguousarray(wk16[:, i * D:(i + 1) * D]),
            "wv": np.ascontiguousarray(wv16[:, i * D:(i + 1) * D]),
            "wo": np.ascontiguousarray(wo16[i * DQ:(i + 1) * DQ, :]),
            "cosT": cosT, "sinT": sinT, "tri": tri, "ones": ones,
            "ident": ident,
        })

    res = run_bass_kernel_spmd(nc, in_maps, core_ids=list(range(NC)), **RUN_KWARGS)
    _BUILD_CACHE["last_res"] = res
    full = np.empty((T, HID), dtype=np.float32)
    for c in range(TC - 1):
        for i in range(NC):
            shard = np.asarray(res.results[i]["out"][c * 64:(c + 1) * 64, :],
                               dtype=np.float32)
            full[c * 512 + i * 64: c * 512 + (i + 1) * 64, :] = shard
    c = TC - 1
    for u in range(4):  # last chunk: per-t-tile 16-row shards
        for i in range(NC):
            shard = np.asarray(
                res.results[i]["out"][c * 64 + u * 16: c * 64 + (u + 1) * 16, :],
                dtype=np.float32)
            base = c * 512 + u * 128 + i * 16
            full[base: base + 16, :] = shard
    return full.reshape(B, T, HID)


if __name__ == "__main__":
    rng = np.random.default_rng(0)
    s = 1.0 / math.sqrt(HID)
    x = rng.standard_normal((1, T, HID), dtype=np.float32)
    wq_ = rng.standard_normal((HID, H * D), dtype=np.float32) * s
    wk_ = rng.standard_normal((HID, KV * D), dtype=np.float32) * s
    wv_ = rng.standard_normal((HID, KV * D), dtype=np.float32) * s
    wo_ = rng.standard_normal((H * D, HID), dtype=np.float32) * s
    o = kernel(x, wq_, wk_, wv_, wo_, None)
    print("out", o.shape, o.dtype, float(np.abs(o).mean()))
